# revision 5
# baseline (speedup 1.0000x reference)
"""Trainium2 Bass kernel for nn_ComputeEdgeLoss.

Computes, for each batch b and lower-triangular pair (i, j) of the 64
recon keypoints, the mean over 5 interpolated segment points of the min
squared distance to the 2048 gt points of that batch.

Strategy (v2)
-------------
Sharding: 8 cores = 4 batches x 2 row-shards; gt replicated per batch.
Per batch the distinct query rows are 3*2016 interior interp points
(f = .25/.5/.75; f = 0/1 rows are the 64 shared endpoints) + 64
endpoint rows = 6112.  The even core of each batch takes 1002 pairs +
the 64 endpoint rows (3070 rows), the odd core 1014 pairs (3042 rows):
both fit 24 row-tiles of 128, the PE floor for this problem
(output-element bound: 24*4*512 cols at 1 col/cycle @1.2GHz = 41 us).

Math: ||k - g||^2 = a . b with a = [kx,ky,kz,||k||^2,1],
b = [-2gx,-2gy,-2gz,1,||g||^2]; fp32 inputs are split host-side into
bf16 h+l+r terms and the 6 product groups >= 2^-24 (hh|hl lh|hr rh ll)
form one K=30 bf16 matmul (matmul cost is K-independent).

Drain (the v2 change): per [128 x 2048] PSUM tile, ScalarE casts half0
to fp16 SBUF (~1.11 us) and ONE custom DVE op (min body + fused min
accumulate, reading PSUM half1 + the casted SBUF half) produces the
[128,1] row-min directly (~1.22 us).  Both sit well under the 1.71 us
PE tile cadence, so the PE never stalls and the tail after the last
matmul is ~2 DVE/Scalar ops + one small DMA.

Input DMA is ordered [gt | first PF tiles | rest] so the first matmul
only waits on the first piece; output BMINS cols drain in two pieces so
only a [128,4] DMA remains after the last tile.
"""

import numpy as np

import concourse.bass as bass
import concourse.mybir as mybir
import concourse.tile as tile
from concourse.bass_utils import run_bass_kernel_spmd

# Problem shape (hardcoded per contest rules).
B = 4          # batches
NPTS = 64      # recon points per batch
M = 2048       # gt points per batch
P = NPTS * (NPTS - 1) // 2   # 2016 pairs
N_CORES = 8
FRACS = (0.25, 0.5, 0.75)    # interior interpolation fractions
NF = len(FRACS)

PAIRS_EVEN = 1002            # even core: pairs + 64 endpoint rows = 3070
PAIRS_ODD = P - PAIRS_EVEN   # 1014 -> 3042 rows
NTILES = 24
ROWS = NTILES * 128          # 3072 rows per core (padded)
KEXT = 30                    # contraction: 6 bf16 product groups x 5
GT_CHUNK = 512               # PSUM bank free size (fp32)
NCHUNK = M // GT_CHUNK       # 4 matmul chunks per row-tile
HM = M // 2                  # 1024: drain half-tile

OUT_SPLIT = 20               # BMINS cols [0:20] DMA'd early, [20:24] at end

_II, _JJ = np.tril_indices(NPTS, -1)   # pair order matches reference

_COMPUTE_ENGINES = {"PE", "DVE", "Activation", "Pool"}


def _split3_bf16(x: np.ndarray):
    """Split fp32 x into three bf16 terms with x ~= h + l + r (27-bit
    significand fidelity; differences are Sterbenz-exact in fp32)."""
    import ml_dtypes

    bf16 = ml_dtypes.bfloat16
    x = np.ascontiguousarray(x, dtype=np.float32)
    h = x.astype(bf16)
    l32 = (x - h.astype(np.float32)).astype(np.float32)
    l = l32.astype(bf16)
    r = (l32 - l.astype(np.float32)).astype(np.float32).astype(bf16)
    return h, l, r


def _register_min_reduce_op():
    """Register a custom DVE op: out = min(in0, in1) elementwise, with a
    fused min-accumulate over the free dim into accum_out (init = s0).

    One such op drains a whole [128 x 2048] distance tile: in0 = PSUM
    half1 (fp32), in1 = the ScalarE-casted fp16 copy of half0 in SBUF
    (the HW allows only one PSUM source per instruction)."""
    import concourse.dve_ops as dops
    from concourse.dve_spec import C0, Spec, Src0, Src1, lower, minn
    from concourse.dve_uop import DveOpSpec

    name = "ANT_TT_MIN_REDUCE_EDGE"
    for o in dops.OPS:
        if o.name == name:
            return o

    def _ref(in0, in1, c0, c1, c2):
        return np.minimum(in0.astype(np.float32), in1.astype(np.float32))

    spec = Spec(body=minn(Src0, Src1), accum=minn, accum_init=C0, reference=_ref)
    row = max(dops._SUB_OPCODE_FOR_NAME.values()) + 1
    assert row < 0x20
    ver = "v3"  # TRN2
    sha = DveOpSpec(
        name=name, opcode=row, uops=lower(spec, ver=ver), rd1_en=True
    ).sha(ver)
    op = dops.DveOp(name, spec, subdim=False, uops_sha={ver: sha})
    dops.OPS.append(op)
    dops.CUSTOM_DVE_SPECS[name] = spec
    dops._SUB_OPCODE_FOR_NAME[name] = row
    return op


def _prune_redundant_waits(bir: dict) -> dict:
    """Reduce every instruction to at most ONE sync-wait.

    This walrus build accepts only one sync-wait per instruction, but
    Tile's semaphore pass is not transitively minimal.  We reconstruct
    per-instruction guaranteed semaphore lower bounds (vector clocks
    over the scheduled program order) and delete implied waits; any
    residual multi-wait instruction is split into single-wait Drain
    carriers on the same engine.

    Soundness model: per-engine in-order dispatch; in-order completion
    for compute engines; per-semaphore in-order completion for DMA-queue
    sems (each DMAHW sem belongs to one queue).  Only monotone
    (inc-only) semaphores with sem-ge-imm waits participate.
    """
    fn = bir["functions"][0]

    contrib_engines: dict[int, set] = {}
    monotone: dict[int, bool] = {}
    for b in fn["blocks"]:
        for ins in b["instructions"]:
            sy = ins.get("sync_info") or {}
            for u in sy.get("on_update") or []:
                if u.get("sync_type") != "semaphore":
                    continue
                s = u["id"]
                contrib_engines.setdefault(s, set()).add(ins.get("engine"))
                ok = u.get("update_mode") == "sem-inc"
                monotone[s] = monotone.get(s, True) and ok

    def usable(s):
        return monotone.get(s, False)

    def mergemax(dst, src):
        for k, v in src.items():
            if dst.get(k, -1) < v:
                dst[k] = v

    prev_start_know: dict[str, dict] = {}
    cum: dict[int, int] = {}            # sem -> cumulative inc in walk order
    comp_know: list[dict] = []          # per walk index
    sem_reach: dict[int, list] = {}     # sem -> [(value_after, walk_idx)]
    dropped = 0
    walk_idx = 0

    for b in fn["blocks"]:
        new_insts = []
        for ins in b["instructions"]:
            eng = ins.get("engine")
            sy = ins.get("sync_info") or {}
            waits = list(sy.get("on_wait") or [])

            def know_from(wlist):
                know = dict(prev_start_know.get(eng, {}))
                for w in wlist:
                    if (w.get("sync_type") != "semaphore"
                            or w.get("wait_mode") != "sem-ge-imm"):
                        continue
                    s, v = w["id"], w["wait_value"]
                    if not usable(s):
                        continue
                    if know.get(s, -1) < v:
                        know[s] = v
                    if len(contrib_engines.get(s, ())) == 1:
                        for after, pidx in sem_reach.get(s, ()):
                            if after >= v:
                                mergemax(know, comp_know[pidx])
                                break
                return know

            if len(waits) > 1:
                kept = list(waits)
                changed = True
                while changed and len(kept) > 1:
                    changed = False
                    for w in list(kept):
                        others = [x for x in kept if x is not w]
                        if (w.get("sync_type") == "semaphore"
                                and w.get("wait_mode") == "sem-ge-imm"
                                and usable(w["id"])
                                and know_from(others).get(w["id"], -1)
                                >= w["wait_value"]):
                            kept.remove(w)
                            dropped += 1
                            changed = True
                            break
                if len(kept) > 1:
                    for k, w in enumerate(kept[:-1]):
                        new_insts.append({
                            "name": f"{ins['name']}-w{k}",
                            "engine": eng, "ins": [], "outs": [],
                            "opcode": "Drain",
                            "sync_info": {"on_wait": [w], "on_update": []},
                        })
                        walk_idx += 1
                        comp_know.append(dict(prev_start_know.get(eng, {})))
                    kept = kept[-1:]
                if len(kept) != len(waits):
                    if not sy:
                        ins["sync_info"] = sy = {"on_update": []}
                    sy["on_wait"] = kept
                    waits = kept

            start_know = know_from(waits)
            prev_start_know[eng] = start_know

            own = set()
            for u in sy.get("on_update") or []:
                if (u.get("sync_type") == "semaphore"
                        and u.get("update_mode") == "sem-inc"):
                    s = u["id"]
                    cum[s] = cum.get(s, 0) + u.get("update_value", 1)
                    sem_reach.setdefault(s, []).append((cum[s], walk_idx))
                    own.add(s)
            ck = dict(start_know)
            for s in own:
                if usable(s) and len(contrib_engines.get(s, ())) == 1:
                    if ck.get(s, -1) < cum[s]:
                        ck[s] = cum[s]
            if eng in _COMPUTE_ENGINES:
                for s, c in cum.items():
                    if (usable(s) and contrib_engines.get(s) == {eng}
                            and ck.get(s, -1) < c):
                        ck[s] = c
            comp_know.append(ck)
            new_insts.append(ins)
            walk_idx += 1
        b["instructions"] = new_insts
    return bir


def _build_nc() -> bass.Bass:
    minred = _register_min_reduce_op()

    nc = bass.Bass()
    # Fused input (gt cols first, then 24 PF tiles) so each matmul/LDW
    # depends on exactly one DMA semaphore.
    ab = nc.declare_dram_parameter("ab", [KEXT, M + ROWS], mybir.dt.bfloat16,
                                   isOutput=False)
    res = nc.declare_dram_parameter("res", [128, NTILES], mybir.dt.float32,
                                    isOutput=True)

    f32 = mybir.dt.float32
    f16 = mybir.dt.float16

    with tile.TileContext(nc) as tc:
        with (
            tc.tile_pool(name="const", bufs=1) as const_pool,
            tc.tile_pool(name="psA", bufs=2, space="PSUM") as psA_pool,
            tc.tile_pool(name="psB", bufs=2, space="PSUM") as psB_pool,
            tc.tile_pool(name="cast", bufs=2) as cast_pool,
        ):
            AB = const_pool.tile([KEXT, M + ROWS], mybir.dt.bfloat16, name="AB")
            JK = const_pool.tile([128, HM], f32, name="JK")
            BM = const_pool.tile([128, NTILES], f32, name="BM")

            # Input DMA: first piece covers gt + PF tiles 0-1 so the first
            # LDW/matmul only waits on it; the rest streams behind compute.
            cut1 = M + 2 * 128
            cut2 = M + 13 * 128
            nc.sync.dma_start(out=AB[:, 0:cut1], in_=ab[:, 0:cut1])
            nc.sync.dma_start(out=AB[:, cut1:cut2], in_=ab[:, cut1:cut2])
            nc.sync.dma_start(out=AB[:, cut2:], in_=ab[:, cut2:])
            GT = AB[:, 0:M]
            PF = AB[:, M:M + ROWS]

            for t in range(NTILES):
                lhsT = PF[:, t * 128:(t + 1) * 128]
                # Two half-tile PSUM buffers per tile: the cast half frees
                # as soon as ScalarE is done with it, so tile t+2's first
                # matmuls never wait on tile t's DVE drain.
                pa = psA_pool.tile([128, HM], f32, tag="pa", name="pa")
                pb = psB_pool.tile([128, HM], f32, tag="pb", name="pb")
                for c in range(NCHUNK):
                    half, sl = divmod(c * GT_CHUNK, HM)
                    dst = pa if half == 0 else pb
                    nc.tensor.matmul(
                        out=dst[:, sl:sl + GT_CHUNK],
                        lhsT=lhsT, rhs=GT[:, c * GT_CHUNK:(c + 1) * GT_CHUNK],
                        start=True, stop=True,
                    )
                # drain: ScalarE casts half0 (ready after chunks 0-1) to
                # fp16; one DVE custom op mins PSUM half1 against it with
                # fused min-accumulate into this tile's BMINS column.
                sc = cast_pool.tile([128, HM], f16, tag="sc", name="sc")
                nc.scalar.copy(sc[:, :], pa[:, :])
                nc.vector._custom_dve(
                    minred, out=JK[:, :], in0=pb[:, :], in1=sc[:, :],
                    s0=3.0e38, accum_out=BM[:, t:t + 1],
                )
                if t == OUT_SPLIT - 1:
                    nc.sync.dma_start(out=res[:, 0:OUT_SPLIT],
                                      in_=BM[:, 0:OUT_SPLIT])
            nc.sync.dma_start(out=res[:, OUT_SPLIT:], in_=BM[:, OUT_SPLIT:])

    import json as _json

    from concourse.library_overlay import lower_extended_insts

    lower_extended_insts(nc)
    pruned = _prune_redundant_waits(_json.loads(nc.to_json_bytes()))
    blob = _json.dumps(pruned).encode()
    nc.to_json_bytes = lambda: blob  # instance override read by bass2jax
    return nc


def _core_pairs(core: int):
    b, h = divmod(core, 2)
    if h == 0:
        return b, _II[:PAIRS_EVEN], _JJ[:PAIRS_EVEN], True
    return b, _II[PAIRS_EVEN:], _JJ[PAIRS_EVEN:], False


def _host_prep(recon_points: np.ndarray, gt_points: np.ndarray):
    """Build per-core [KEXT, M + ROWS] fused bf16 operand."""
    in_maps = []
    for core in range(N_CORES):
        b, ii, jj, has_ep = _core_pairs(core)
        rec = recon_points[b].astype(np.float32)          # [64, 3]
        start, end = rec[ii], rec[jj]                     # [np, 3]
        npair = len(ii)

        A = np.zeros((5, ROWS), dtype=np.float32)
        for fi, f in enumerate(FRACS):
            k = (start * np.float32(f) + end * np.float32(1.0 - f)).astype(np.float32)
            cols = slice(fi * npair, (fi + 1) * npair)
            A[0:3, cols] = k.T
            A[3, cols] = (k.astype(np.float64) ** 2).sum(1).astype(np.float32)
            A[4, cols] = 1.0
        if has_ep:
            ep = slice(NF * npair, NF * npair + NPTS)
            A[0:3, ep] = rec.T
            A[3, ep] = (rec.astype(np.float64) ** 2).sum(1).astype(np.float32)
            A[4, ep] = 1.0

        g = gt_points[b].astype(np.float32)               # [2048, 3]
        Bm = np.empty((5, M), dtype=np.float32)
        Bm[0:3] = np.float32(-2.0) * g.T
        Bm[3] = 1.0
        Bm[4] = (g.astype(np.float64) ** 2).sum(1).astype(np.float32)

        Ah, Al, Ar = _split3_bf16(A)
        Bh, Bl, Br = _split3_bf16(Bm)
        # Product groups, largest magnitude first: hh | hl lh | hr rh ll
        A_ext = np.concatenate([Ah, Ah, Al, Ah, Ar, Al], axis=0)  # [30, ROWS]
        B_ext = np.concatenate([Bh, Bl, Bh, Br, Bh, Bl], axis=0)  # [30, M]
        ab = np.concatenate([B_ext, A_ext], axis=1)       # [30, M + ROWS]
        in_maps.append({"ab": np.ascontiguousarray(ab)})
    return in_maps


def _host_assemble(results) -> np.ndarray:
    out = np.empty((B, P), dtype=np.float32)
    ep_mins = {}
    core_mins = []
    for core in range(N_CORES):
        res = np.asarray(results[core]["res"], dtype=np.float32)
        mins = res.T.reshape(-1)          # row r of core = mins[r]
        core_mins.append(mins)
        b, ii, jj, has_ep = _core_pairs(core)
        if has_ep:
            npair = len(ii)
            ep_mins[b] = mins[NF * npair:NF * npair + NPTS]
    for core in range(N_CORES):
        b, ii, jj, has_ep = _core_pairs(core)
        mins = core_mins[core]
        npair = len(ii)
        s3 = mins[0:npair] + mins[npair:2 * npair] + mins[2 * npair:3 * npair]
        E = ep_mins[b]
        vals = (s3 + E[ii] + E[jj]) * np.float32(0.2)
        if has_ep:
            out[b, :PAIRS_EVEN] = vals
        else:
            out[b, PAIRS_EVEN:] = vals
    return out


_NC_CACHE = None


def _get_nc() -> bass.Bass:
    global _NC_CACHE
    if _NC_CACHE is None:
        _NC_CACHE = _build_nc()
    return _NC_CACHE


def run(recon_points: np.ndarray, gt_points: np.ndarray, **spmd_kwargs):
    """Run on 8 NeuronCores; returns (output [4, 2016], BassKernelResults)."""
    nc = _get_nc()
    in_maps = _host_prep(recon_points, gt_points)
    r = run_bass_kernel_spmd(nc, in_maps, list(range(N_CORES)), **spmd_kwargs)
    return _host_assemble(r.results), r


def kernel(recon_points: np.ndarray, gt_points: np.ndarray) -> np.ndarray:
    recon_points = np.asarray(recon_points, dtype=np.float32)
    gt_points = np.asarray(gt_points, dtype=np.float32)
    out, _ = run(recon_points, gt_points)
    return out


# revision 6
# speedup vs baseline: 1.0944x; 1.0944x over previous
"""Trainium2 Bass kernel for nn_ComputeEdgeLoss.

Computes, for each batch b and lower-triangular pair (i, j) of the 64
recon keypoints, the mean over 5 interpolated segment points of the min
squared distance to the 2048 gt points of that batch.

Strategy (v2)
-------------
Sharding: 8 cores = 4 batches x 2 row-shards; gt replicated per batch.
Per batch the distinct query rows are 3*2016 interior interp points
(f = .25/.5/.75; f = 0/1 rows are the 64 shared endpoints) + 64
endpoint rows = 6112.  The even core of each batch takes 1002 pairs +
the 64 endpoint rows (3070 rows), the odd core 1014 pairs (3042 rows):
both fit 24 row-tiles of 128, the PE floor for this problem
(output-element bound: 24*4*512 cols at 1 col/cycle @1.2GHz = 41 us).

Math: ||k - g||^2 = a . b with a = [kx,ky,kz,||k||^2,1],
b = [-2gx,-2gy,-2gz,1,||g||^2]; fp32 inputs are split host-side into
bf16 h+l+r terms and the 6 product groups >= 2^-24 (hh|hl lh|hr rh ll)
form one K=30 bf16 matmul (matmul cost is K-independent).

Drain (the v2 change): per [128 x 2048] PSUM tile, ScalarE casts half0
to fp16 SBUF (~1.11 us) and ONE custom DVE op (min body + fused min
accumulate, reading PSUM half1 + the casted SBUF half) produces the
[128,1] row-min directly (~1.22 us).  Both sit well under the 1.71 us
PE tile cadence, so the PE never stalls and the tail after the last
matmul is ~2 DVE/Scalar ops + one small DMA.

Input DMA is ordered [gt | first PF tiles | rest] so the first matmul
only waits on the first piece; output BMINS cols drain in two pieces so
only a [128,4] DMA remains after the last tile.
"""

import numpy as np

import concourse.bass as bass
import concourse.mybir as mybir
import concourse.tile as tile
from concourse.bass_utils import run_bass_kernel_spmd

# Problem shape (hardcoded per contest rules).
B = 4          # batches
NPTS = 64      # recon points per batch
M = 2048       # gt points per batch
P = NPTS * (NPTS - 1) // 2   # 2016 pairs
N_CORES = 8
FRACS = (0.25, 0.5, 0.75)    # interior interpolation fractions
NF = len(FRACS)

PAIRS_EVEN = 1002            # even core: pairs + 64 endpoint rows = 3070
PAIRS_ODD = P - PAIRS_EVEN   # 1014 -> 3042 rows
NTILES = 24
ROWS = NTILES * 128          # 3072 rows per core (padded)
KEXT = 30                    # contraction: 6 bf16 product groups x 5
GT_CHUNK = 512               # PSUM bank free size (fp32)
NCHUNK = M // GT_CHUNK       # 4 matmul chunks per row-tile
HM = M // 2                  # 1024: drain half-tile

OUT_SPLIT = 20               # BMINS cols [0:20] DMA'd early, [20:24] at end

_II, _JJ = np.tril_indices(NPTS, -1)   # pair order matches reference

_COMPUTE_ENGINES = {"PE", "DVE", "Activation", "Pool"}


def _split3_bf16(x: np.ndarray):
    """Split fp32 x into three bf16 terms with x ~= h + l + r (27-bit
    significand fidelity; differences are Sterbenz-exact in fp32)."""
    import ml_dtypes

    bf16 = ml_dtypes.bfloat16
    x = np.ascontiguousarray(x, dtype=np.float32)
    h = x.astype(bf16)
    l32 = (x - h.astype(np.float32)).astype(np.float32)
    l = l32.astype(bf16)
    r = (l32 - l.astype(np.float32)).astype(np.float32).astype(bf16)
    return h, l, r


def _register_min_reduce_op():
    """Register a custom DVE op: out = min(in0, in1) elementwise, with a
    fused min-accumulate over the free dim into accum_out (init = s0).

    One such op drains a whole [128 x 2048] distance tile: in0 = PSUM
    half1 (fp32), in1 = the ScalarE-casted fp16 copy of half0 in SBUF
    (the HW allows only one PSUM source per instruction)."""
    import concourse.dve_ops as dops
    from concourse.dve_spec import C0, Spec, Src0, Src1, lower, minn
    from concourse.dve_uop import DveOpSpec

    name = "ANT_TT_MIN_REDUCE_EDGE"
    for o in dops.OPS:
        if o.name == name:
            return o

    def _ref(in0, in1, c0, c1, c2):
        return np.minimum(in0.astype(np.float32), in1.astype(np.float32))

    spec = Spec(body=minn(Src0, Src1), accum=minn, accum_init=C0, reference=_ref)
    row = max(dops._SUB_OPCODE_FOR_NAME.values()) + 1
    assert row < 0x20
    ver = "v3"  # TRN2
    sha = DveOpSpec(
        name=name, opcode=row, uops=lower(spec, ver=ver), rd1_en=True
    ).sha(ver)
    op = dops.DveOp(name, spec, subdim=False, uops_sha={ver: sha})
    dops.OPS.append(op)
    dops.CUSTOM_DVE_SPECS[name] = spec
    dops._SUB_OPCODE_FOR_NAME[name] = row
    return op


def _prune_redundant_waits(bir: dict) -> dict:
    """Reduce every instruction to at most ONE sync-wait.

    This walrus build accepts only one sync-wait per instruction, but
    Tile's semaphore pass is not transitively minimal.  We reconstruct
    per-instruction guaranteed semaphore lower bounds (vector clocks
    over the scheduled program order) and delete implied waits; any
    residual multi-wait instruction is split into single-wait Drain
    carriers on the same engine.

    Soundness model: per-engine in-order dispatch; in-order completion
    for compute engines; per-semaphore in-order completion for DMA-queue
    sems (each DMAHW sem belongs to one queue).  Only monotone
    (inc-only) semaphores with sem-ge-imm waits participate.
    """
    fn = bir["functions"][0]

    contrib_engines: dict[int, set] = {}
    monotone: dict[int, bool] = {}
    for b in fn["blocks"]:
        for ins in b["instructions"]:
            sy = ins.get("sync_info") or {}
            for u in sy.get("on_update") or []:
                if u.get("sync_type") != "semaphore":
                    continue
                s = u["id"]
                contrib_engines.setdefault(s, set()).add(ins.get("engine"))
                ok = u.get("update_mode") == "sem-inc"
                monotone[s] = monotone.get(s, True) and ok

    def usable(s):
        return monotone.get(s, False)

    def mergemax(dst, src):
        for k, v in src.items():
            if dst.get(k, -1) < v:
                dst[k] = v

    prev_start_know: dict[str, dict] = {}
    cum: dict[int, int] = {}            # sem -> cumulative inc in walk order
    comp_know: list[dict] = []          # per walk index
    sem_reach: dict[int, list] = {}     # sem -> [(value_after, walk_idx)]
    dropped = 0
    walk_idx = 0

    for b in fn["blocks"]:
        new_insts = []
        for ins in b["instructions"]:
            eng = ins.get("engine")
            sy = ins.get("sync_info") or {}
            waits = list(sy.get("on_wait") or [])

            def know_from(wlist):
                know = dict(prev_start_know.get(eng, {}))
                for w in wlist:
                    if (w.get("sync_type") != "semaphore"
                            or w.get("wait_mode") != "sem-ge-imm"):
                        continue
                    s, v = w["id"], w["wait_value"]
                    if not usable(s):
                        continue
                    if know.get(s, -1) < v:
                        know[s] = v
                    if len(contrib_engines.get(s, ())) == 1:
                        for after, pidx in sem_reach.get(s, ()):
                            if after >= v:
                                mergemax(know, comp_know[pidx])
                                break
                return know

            if len(waits) > 1:
                kept = list(waits)
                changed = True
                while changed and len(kept) > 1:
                    changed = False
                    for w in list(kept):
                        others = [x for x in kept if x is not w]
                        if (w.get("sync_type") == "semaphore"
                                and w.get("wait_mode") == "sem-ge-imm"
                                and usable(w["id"])
                                and know_from(others).get(w["id"], -1)
                                >= w["wait_value"]):
                            kept.remove(w)
                            dropped += 1
                            changed = True
                            break
                if len(kept) > 1:
                    for k, w in enumerate(kept[:-1]):
                        new_insts.append({
                            "name": f"{ins['name']}-w{k}",
                            "engine": eng, "ins": [], "outs": [],
                            "opcode": "Drain",
                            "sync_info": {"on_wait": [w], "on_update": []},
                        })
                        walk_idx += 1
                        comp_know.append(dict(prev_start_know.get(eng, {})))
                    kept = kept[-1:]
                if len(kept) != len(waits):
                    if not sy:
                        ins["sync_info"] = sy = {"on_update": []}
                    sy["on_wait"] = kept
                    waits = kept

            start_know = know_from(waits)
            prev_start_know[eng] = start_know

            own = set()
            for u in sy.get("on_update") or []:
                if (u.get("sync_type") == "semaphore"
                        and u.get("update_mode") == "sem-inc"):
                    s = u["id"]
                    cum[s] = cum.get(s, 0) + u.get("update_value", 1)
                    sem_reach.setdefault(s, []).append((cum[s], walk_idx))
                    own.add(s)
            ck = dict(start_know)
            for s in own:
                if usable(s) and len(contrib_engines.get(s, ())) == 1:
                    if ck.get(s, -1) < cum[s]:
                        ck[s] = cum[s]
            if eng in _COMPUTE_ENGINES:
                for s, c in cum.items():
                    if (usable(s) and contrib_engines.get(s) == {eng}
                            and ck.get(s, -1) < c):
                        ck[s] = c
            comp_know.append(ck)
            new_insts.append(ins)
            walk_idx += 1
        b["instructions"] = new_insts
    return bir


def _build_nc() -> bass.Bass:
    minred = _register_min_reduce_op()

    nc = bass.Bass()
    # Fused input (gt cols first, then 24 PF tiles) so each matmul/LDW
    # depends on exactly one DMA semaphore.
    ab = nc.declare_dram_parameter("ab", [KEXT, M + ROWS], mybir.dt.bfloat16,
                                   isOutput=False)
    res = nc.declare_dram_parameter("res", [128, NTILES], mybir.dt.float32,
                                    isOutput=True)

    f32 = mybir.dt.float32
    f16 = mybir.dt.float16

    with tile.TileContext(nc) as tc:
        with (
            tc.tile_pool(name="const", bufs=1) as const_pool,
            tc.tile_pool(name="ps", bufs=1, space="PSUM") as ps_pool,
            tc.tile_pool(name="cast", bufs=1) as cast_pool,
        ):
            AB = const_pool.tile([KEXT, M + ROWS], mybir.dt.bfloat16, name="AB")
            JK = const_pool.tile([128, HM], f32, name="JK")
            BM = const_pool.tile([128, NTILES], f32, name="BM")
            PA = [ps_pool.tile([128, HM], f32, name=f"pa{k}", tag=f"pa{k}")
                  for k in range(2)]
            PB = [ps_pool.tile([128, HM], f32, name=f"pb{k}", tag=f"pb{k}")
                  for k in range(2)]
            SCB = [cast_pool.tile([128, HM], f16, name=f"sc{k}", tag=f"sc{k}")
                   for k in range(2)]

            # Input DMA: first piece covers gt + PF tiles 0-1 so the first
            # LDW/matmul only waits on it; the rest streams behind compute.
            cut1 = M + 2 * 128
            cut2 = M + 13 * 128
            nc.sync.dma_start(out=AB[:, 0:cut1], in_=ab[:, 0:cut1])
            nc.sync.dma_start(out=AB[:, cut1:cut2], in_=ab[:, cut1:cut2])
            nc.sync.dma_start(out=AB[:, cut2:], in_=ab[:, cut2:])
            GT = AB[:, 0:M]
            PF = AB[:, M:M + ROWS]

            for t in range(NTILES):
                lhsT = PF[:, t * 128:(t + 1) * 128]
                # Two half-tile PSUM buffers per tile, manually rotated
                # (t%2): exact WAR deps, no pool-rotation artifacts.
                pa = PA[t % 2]
                pb = PB[t % 2]
                for c in range(NCHUNK):
                    half, sl = divmod(c * GT_CHUNK, HM)
                    dst = pa if half == 0 else pb
                    nc.tensor.matmul(
                        out=dst[:, sl:sl + GT_CHUNK],
                        lhsT=lhsT, rhs=GT[:, c * GT_CHUNK:(c + 1) * GT_CHUNK],
                        start=True, stop=True,
                    )
                # drain: ScalarE casts half0 (ready after chunks 0-1) to
                # fp16; one DVE custom op mins PSUM half1 against it with
                # fused min-accumulate into this tile's BMINS column.
                sc = SCB[t % 2]
                nc.scalar.copy(sc[:, :], pa[:, :])
                nc.vector._custom_dve(
                    minred, out=JK[:, :], in0=pb[:, :], in1=sc[:, :],
                    s0=3.0e38, accum_out=BM[:, t:t + 1],
                )
                if t == OUT_SPLIT - 1:
                    nc.sync.dma_start(out=res[:, 0:OUT_SPLIT],
                                      in_=BM[:, 0:OUT_SPLIT])
            nc.sync.dma_start(out=res[:, OUT_SPLIT:], in_=BM[:, OUT_SPLIT:])

    import json as _json

    from concourse.library_overlay import lower_extended_insts

    lower_extended_insts(nc)
    pruned = _prune_redundant_waits(_json.loads(nc.to_json_bytes()))
    blob = _json.dumps(pruned).encode()
    nc.to_json_bytes = lambda: blob  # instance override read by bass2jax
    return nc


def _core_pairs(core: int):
    b, h = divmod(core, 2)
    if h == 0:
        return b, _II[:PAIRS_EVEN], _JJ[:PAIRS_EVEN], True
    return b, _II[PAIRS_EVEN:], _JJ[PAIRS_EVEN:], False


def _host_prep(recon_points: np.ndarray, gt_points: np.ndarray):
    """Build per-core [KEXT, M + ROWS] fused bf16 operand."""
    in_maps = []
    for core in range(N_CORES):
        b, ii, jj, has_ep = _core_pairs(core)
        rec = recon_points[b].astype(np.float32)          # [64, 3]
        start, end = rec[ii], rec[jj]                     # [np, 3]
        npair = len(ii)

        A = np.zeros((5, ROWS), dtype=np.float32)
        for fi, f in enumerate(FRACS):
            k = (start * np.float32(f) + end * np.float32(1.0 - f)).astype(np.float32)
            cols = slice(fi * npair, (fi + 1) * npair)
            A[0:3, cols] = k.T
            A[3, cols] = (k.astype(np.float64) ** 2).sum(1).astype(np.float32)
            A[4, cols] = 1.0
        if has_ep:
            ep = slice(NF * npair, NF * npair + NPTS)
            A[0:3, ep] = rec.T
            A[3, ep] = (rec.astype(np.float64) ** 2).sum(1).astype(np.float32)
            A[4, ep] = 1.0

        g = gt_points[b].astype(np.float32)               # [2048, 3]
        Bm = np.empty((5, M), dtype=np.float32)
        Bm[0:3] = np.float32(-2.0) * g.T
        Bm[3] = 1.0
        Bm[4] = (g.astype(np.float64) ** 2).sum(1).astype(np.float32)

        Ah, Al, Ar = _split3_bf16(A)
        Bh, Bl, Br = _split3_bf16(Bm)
        # Product groups, largest magnitude first: hh | hl lh | hr rh ll
        A_ext = np.concatenate([Ah, Ah, Al, Ah, Ar, Al], axis=0)  # [30, ROWS]
        B_ext = np.concatenate([Bh, Bl, Bh, Br, Bh, Bl], axis=0)  # [30, M]
        ab = np.concatenate([B_ext, A_ext], axis=1)       # [30, M + ROWS]
        in_maps.append({"ab": np.ascontiguousarray(ab)})
    return in_maps


def _host_assemble(results) -> np.ndarray:
    out = np.empty((B, P), dtype=np.float32)
    ep_mins = {}
    core_mins = []
    for core in range(N_CORES):
        res = np.asarray(results[core]["res"], dtype=np.float32)
        mins = res.T.reshape(-1)          # row r of core = mins[r]
        core_mins.append(mins)
        b, ii, jj, has_ep = _core_pairs(core)
        if has_ep:
            npair = len(ii)
            ep_mins[b] = mins[NF * npair:NF * npair + NPTS]
    for core in range(N_CORES):
        b, ii, jj, has_ep = _core_pairs(core)
        mins = core_mins[core]
        npair = len(ii)
        s3 = mins[0:npair] + mins[npair:2 * npair] + mins[2 * npair:3 * npair]
        E = ep_mins[b]
        vals = (s3 + E[ii] + E[jj]) * np.float32(0.2)
        if has_ep:
            out[b, :PAIRS_EVEN] = vals
        else:
            out[b, PAIRS_EVEN:] = vals
    return out


_NC_CACHE = None


def _get_nc() -> bass.Bass:
    global _NC_CACHE
    if _NC_CACHE is None:
        _NC_CACHE = _build_nc()
    return _NC_CACHE


def run(recon_points: np.ndarray, gt_points: np.ndarray, **spmd_kwargs):
    """Run on 8 NeuronCores; returns (output [4, 2016], BassKernelResults)."""
    nc = _get_nc()
    in_maps = _host_prep(recon_points, gt_points)
    r = run_bass_kernel_spmd(nc, in_maps, list(range(N_CORES)), **spmd_kwargs)
    return _host_assemble(r.results), r


def kernel(recon_points: np.ndarray, gt_points: np.ndarray) -> np.ndarray:
    recon_points = np.asarray(recon_points, dtype=np.float32)
    gt_points = np.asarray(gt_points, dtype=np.float32)
    out, _ = run(recon_points, gt_points)
    return out
